# revision 33
# baseline (speedup 1.0000x reference)
"""Trainium2 Bass kernel for nn_EncodingShake (VQ codebook encoding with shake).

Math (per batch b, n = flattened H*W):
  P[n,k]   = -2 s_k <x_n, c_k> + (s_k - smax) x2[n] + s_k c2[k]
           = sl[n,k] - smax*x2[n]            (per-n shift cancels in softmax)
  A        = softmax_k(P) = exp(P) / sum_k exp(P)
  E[k,d]   = sum_n A[n,k] x_n[d] - (sum_n A[n,k]) C[k,d]

Sharding: data-parallel over B - 8 cores x 2 batches each; codebook/scale
replicated; no collectives.

Device pipeline (per batch, slabs of 4 n-tiles x 128 rows):
  PE  : per tile, 5-matmul accumulation chain into PSL[128,4,32]:
          4 chunk-matmuls  X_c^T @ rsl_c   (rsl = -2 s C^T, bf16)
        + rank-2 mini-matmul [x2;1]^T @ [s-smax; s*c2]  (x2 host-computed)
        per tile, 4 bf16 is_transpose matmuls -> PXT[128,512] (X^T, bf16 psum)
  ACT : expsl = exp(PSL) per slab [128,4,32] psum->sbuf bf16
  DVE : den = group-reduce(expsl) [128,4]; r = 1/den; esc = expsl * bcast(r)
  DVE/ACT/Pool (round-robin): copy PXT -> XT sbuf tiles
  PE  : E^T chunk chains: PET[:,c,:] += XT[:,c]^T @ esc   (4x 32-wide)
        CS chain: PCS[32,1] += esc^T @ ones
  finalize: transpose E^T back via PE, e = PET^T + PCS*(-C), DMA out.

All GEMMs bf16 (fp32 matmul is 4 cycles/row vs bf16's 1); X uploaded bf16
(halves HBM traffic); x2 folded into the GEMM (kills per-tile vector work).
"""

import numpy as np
import ml_dtypes

import bass_rust
import concourse.bass as bass
import concourse.mybir as mybir
import concourse.tile as tile
from concourse.masks import make_identity

# ---------------------------------------------------------------------------
# problem constants (hardcoded per contract)
B, D, H, W, K = 16, 512, 60, 60, 32
N = H * W  # 3600
N_CORES = 8
BPC = B // N_CORES  # batches per core = 2
DC = D // 128  # 4 d-chunks
NT = (N + 127) // 128  # 29 n-tiles (28 x 128 + 1 x 16)
NSLAB = (N + 511) // 512  # 8 slabs (7 full x 4 tiles + 1 x 1 tile of 16)

FP = mybir.dt.float32
BF = mybir.dt.bfloat16
ALU = mybir.AluOpType
ACTF = mybir.ActivationFunctionType

# X DMA piece boundaries (columns), slab-aligned, every piece >= 1823 B so
# each descriptor runs at full DMA bus rate.
X_PIECES = [(0, 1024), (1024, 2048), (2048, 3600)]


def _patched_drain_and_barrier(self, tick_clock, wait_clock):
    # This walrus build accepts only ONE sync wait per instruction; the stock
    # TileContext exit emits a single drain carrying one wait per trailing
    # proc. Split it into a chain of single-wait drains.
    from concourse.vector_clock import ScopedClock

    drain_inst = self.nc.sync.drain()
    wait_clock.add_sem_waits(
        drain_inst.ins, ScopedClock({None: tick_clock.global_clock})
    )
    si = drain_inst.ins.sync_info
    waits = list(si.on_wait) if si is not None else []
    if len(waits) > 1:
        drain_inst.ins.sync_info = bass_rust.SyncInfo(
            on_wait=[waits[0]], on_update=list(si.on_update)
        )
        for w in waits[1:]:
            d2 = self.nc.sync.drain()
            d2.ins.sync_info = bass_rust.SyncInfo(on_wait=[w], on_update=[])
    self.nc.all_engine_barrier()
    assert self.sems is not None
    popped = self.nc._tile_sem_poison_stack.pop()
    assert popped is self._sem_poison
    self.nc.clear_and_free_semaphores(list(self.sems.allocated().values()))
    self.nc.all_engine_barrier()


tile.TileContext._drain_and_barrier = _patched_drain_and_barrier


def _split_multiwaits(obj):
    """Walk BIR JSON; any instruction with >1 on_wait gets the extra waits
    hoisted onto same-engine EventSemaphore carriers inserted before it."""
    counter = [0]

    def fix_list(insts):
        out = []
        for inst in insts:
            si = inst.get("sync_info") if isinstance(inst, dict) else None
            waits = (si or {}).get("on_wait") or []
            if len(waits) > 1:
                for w in waits[:-1]:
                    counter[0] += 1
                    out.append(
                        {
                            "debug": inst.get("debug", 0),
                            "engine": inst["engine"],
                            "ins": [],
                            "name": f"{inst['name']}-smw{counter[0]}",
                            "opcode": "EventSemaphore",
                            "outs": [],
                            "sync_info": {"on_update": [], "on_wait": [w]},
                        }
                    )
                si["on_wait"] = [waits[-1]]
            out.append(inst)
        return out

    def walk(o):
        if isinstance(o, dict):
            for k, v in o.items():
                if k == "instructions" and isinstance(v, list):
                    o[k] = fix_list(v)
                else:
                    walk(v)
        elif isinstance(o, list):
            for v in o:
                walk(v)

    walk(obj)
    return counter[0]


def _install_compile_patch():
    import json as _json

    from concourse import bass2jax, bass_utils

    if getattr(bass2jax, "_smw_patch", False):
        return
    _orig = bass_utils.compile_bir_kernel

    def _patched(bir_json, tmpdir, neff_name="file.neff"):
        d = _json.loads(bir_json)
        n = _split_multiwaits(d)
        if n:
            bir_json = _json.dumps(d).encode()
        return _orig(bir_json, tmpdir, neff_name=neff_name)

    bass2jax.compile_bir_kernel = _patched
    bass2jax._smw_patch = True


_install_compile_patch()


def _slab_tiles(s):
    """(tile indices, rows per tile) for slab s of one batch."""
    t0 = s * 4
    tiles = []
    for ti in range(4):
        t = t0 + ti
        if t >= NT:
            break
        tiles.append((t, min(128, N - t * 128)))
    return tiles


def build(reps: int = 1, stages: str = "full") -> bass.Bass:
    nc = bass.Bass()

    x_d = nc.dram_tensor("x", (BPC, DC, 128, N), BF, kind="ExternalInput")
    aug_d = nc.dram_tensor("aug", (BPC, 2, N), BF, kind="ExternalInput")
    rsl_d = nc.dram_tensor("rsl", (D, K), BF, kind="ExternalInput")
    aug2_d = nc.dram_tensor("aug2", (2, K), BF, kind="ExternalInput")
    cneg_d = nc.dram_tensor("cneg", (K, D), FP, kind="ExternalInput")
    e_d = nc.dram_tensor("e", (BPC, K, D), FP, kind="ExternalOutput")

    with tile.TileContext(nc) as tc:
        with (
            tc.tile_pool(name="singles", bufs=1) as singles,
            tc.tile_pool(name="xpool", bufs=1) as xpool,
            tc.tile_pool(name="xtp", bufs=6) as xtp,
            tc.tile_pool(name="smp", bufs=3) as smp,
            tc.tile_pool(name="denp", bufs=3) as denp,
            tc.tile_pool(name="finp", bufs=2) as finp,
            tc.tile_pool(name="psl_p", bufs=2, space="PSUM") as psl_p,
            tc.tile_pool(name="pxt_p", bufs=4, space="PSUM") as pxt_p,
            tc.tile_pool(name="pet_p", bufs=1, space="PSUM") as pet_p,
            tc.tile_pool(name="pef_p", bufs=1, space="PSUM") as pef_p,
        ):
            ident = singles.tile([128, 128], BF, name="ident")
            make_identity(nc, ident)
            identf = singles.tile([128, 128], FP, name="identf")
            make_identity(nc, identf)

            rsl_sb = singles.tile([128, DC, K], BF, name="rsl_sb")
            nc.scalar.dma_start(
                out=rsl_sb, in_=rsl_d[:, :].rearrange("(c p) k -> p c k", p=128)
            )
            aug2_sb = singles.tile([2, K], BF, name="aug2_sb")
            nc.scalar.dma_start(out=aug2_sb, in_=aug2_d[:, :])
            cneg_sb = singles.tile([K, D], FP, name="cneg_sb")
            nc.scalar.dma_start(out=cneg_sb, in_=cneg_d[:, :])
            ones_sb = singles.tile([128, 1], BF, name="ones_sb")
            nc.vector.memset(ones_sb, 1.0)
            actwarm = singles.tile([128, 1], BF, name="actwarm")
            nc.scalar.activation(
                out=actwarm, in_=ones_sb, func=ACTF.Exp
            )

            def emit_rep():
                # ---- input DMAs: X pieces then aug, batch-major ----
                xin = {}
                aug_sb = {}
                for b in range(BPC):
                    for c in range(DC):
                        xin[(b, c)] = xpool.tile(
                            [128, N], BF, tag=f"x{b}{c}", name=f"x{b}{c}",
                            bufs=2,
                        )
                for b in range(BPC):
                    for (lo, hi) in X_PIECES:
                        for c in range(DC):
                            nc.sync.dma_start(
                                out=xin[(b, c)][:, lo:hi],
                                in_=x_d[b, c, :, lo:hi],
                            )
                    aug_sb[b] = xpool.tile(
                        [2, N], BF, tag=f"aug{b}", name=f"aug{b}", bufs=2
                    )
                    nc.sync.dma_start(out=aug_sb[b], in_=aug_d[b, :, :])

                # round-robin for PXT -> SBUF copies
                copy_cycle = ["pool", "vector", "scalar"]
                copy_i = [0]

                def do_copy(dst, src):
                    eng = copy_cycle[copy_i[0] % len(copy_cycle)]
                    copy_i[0] += 1
                    if eng == "pool":
                        nc.gpsimd.tensor_copy(out=dst, in_=src)
                    elif eng == "vector":
                        nc.vector.tensor_copy(out=dst, in_=src)
                    else:
                        nc.scalar.copy(out=dst, in_=src)

                # per-(b,s) stage state
                steps = [(b, s) for b in range(BPC) for s in range(NSLAB)]
                stE = {}  # (b,s) -> dict with xts, tiles
                psls = {}  # (b,q) -> quad psum
                escs = {}  # (b,q) -> quad esc sbuf

                def quad_slots(q):
                    """number of tile slots in quad q of one batch."""
                    return min(16, NT - q * 16)

                def stage_front(b, s):
                    """PE sl+transpose chains; copies for X^T."""
                    tiles = _slab_tiles(s)
                    q = s // 4
                    if s % 4 == 0:
                        psls[(b, q)] = psl_p.tile(
                            [128, 16, K], FP, tag="psl", name="psl"
                        )
                        if quad_slots(q) % 4 != 0:
                            # zero the partial slot (full partition range -
                            # the BIR verifier rejects partial-partition
                            # writes) so the quad-wide softmax reads defined
                            # data; the sl chain then overwrites rows [:16]
                            nc.vector.memset(
                                psls[(b, q)][:, quad_slots(q) - 1, :], 0.0
                            )
                    psl = psls[(b, q)]
                    xts = None
                    pxts = []
                    pxt = None
                    for ti, (t, nt) in enumerate(tiles):
                        lo = t * 128
                        slot = (s % 4) * 4 + ti
                        for c in range(DC):
                            nc.tensor.matmul(
                                psl[:nt, slot, :],
                                xin[(b, c)][:, lo:lo + nt],
                                rsl_sb[:, c, :],
                                start=(c == 0),
                                stop=False,
                            )
                        nc.tensor.matmul(
                            psl[:nt, slot, :],
                            aug_sb[b][:, lo:lo + nt],
                            aug2_sb[:, :],
                            start=False,
                            stop=True,
                        )
                        if ti % 2 == 0:
                            pxt = pxt_p.tile(
                                [128, 2, 512], BF, tag="pxt", name="pxt"
                            )
                            pxts.append(pxt)
                        for c in range(DC):
                            nc.tensor.matmul(
                                pxt[:nt, ti % 2, c * 128:(c + 1) * 128],
                                xin[(b, c)][:, lo:lo + nt],
                                ident[:, :],
                                start=True,
                                stop=True,
                                is_transpose=True,
                            )
                    if stages in ("full", "front", "nocopy", "cponly"):
                        xts = xtp.tile(
                            [128, 4, 512], BF, tag="xt", name="xt"
                        )
                    stE[(b, s)] = {
                        "xts": xts, "tiles": tiles, "pxts": pxts
                    }

                def emit_copies(b, s):
                    st = stE[(b, s)]
                    xts, tiles, pxts = st["xts"], st["tiles"], st["pxts"]
                    nts = len(tiles)
                    nrow = 128 if nts > 1 else tiles[0][1]
                    for hi in range((nts + 1) // 2):
                        nh = min(2, nts - hi * 2)
                        do_copy(
                            xts[:nrow, hi * 2:hi * 2 + nh, :],
                            pxts[hi][:nrow, :nh, :],
                        )

                def half_softmax(b, q, h):
                    """ACT exp + DVE den/recip/esc for half a quad (8 slots)."""
                    ns = quad_slots(q)
                    lo_s = h * 8
                    hi_s = min(lo_s + 8, ns)
                    w = hi_s - lo_s
                    psl = psls[(b, q)] if h == 0 else psls.pop((b, q))
                    expsl = smp.tile(
                        [128, 8, K], BF, tag="expsl", name="expsl"
                    )
                    nc.scalar.activation(
                        out=expsl[:, :w, :],
                        in_=psl[:, lo_s:hi_s, :],
                        func=ACTF.Exp,
                    )
                    den = denp.tile([128, 8], FP, tag="den", name="den")
                    nc.vector.tensor_reduce(
                        out=den[:, :w],
                        in_=expsl[:, :w, :],
                        axis=mybir.AxisListType.X,
                        op=ALU.add,
                    )
                    rcl = denp.tile([128, 8], BF, tag="rcl", name="rcl")
                    with nc.allow_low_precision(reason="1/den fits bf16"):
                        nc.vector.reciprocal(rcl[:, :w], den[:, :w])
                    if lo_s == 0:
                        escs[(b, q)] = smp.tile(
                            [128, 16, K], BF, tag="esc", name="esc"
                        )
                    esc = escs[(b, q)]
                    rb_full = rcl[:, :w]
                    rb = bass.AP(
                        tensor=rb_full.tensor,
                        offset=rb_full.offset,
                        ap=[list(rb_full.ap[0]), list(rb_full.ap[1]), [0, K]],
                    )
                    nc.vector.tensor_mul(
                        out=esc[:, lo_s:hi_s, :],
                        in0=expsl[:, :w, :],
                        in1=rb,
                    )

                def stage_back(b, s, pet):
                    """PE E^T + CS chains for slab s of batch b."""
                    st = stE.pop((b, s))
                    xts, tiles = st["xts"], st["tiles"]
                    esc = escs[(b, s // 4)]
                    del st
                    for ti, (t, nt) in enumerate(tiles):
                        slot = (s % 4) * 4 + ti
                        # ONE psum accumulation group for the whole bank:
                        # start only on the very first link (clears the
                        # bank's has_written bits; later chains' first
                        # writes then overwrite-on-first-touch), stop only
                        # on the very last link.
                        for c in range(DC):
                            nc.tensor.matmul(
                                pet[:, c * K:(c + 1) * K],
                                xts[:nt, ti, c * 128:(c + 1) * 128],
                                esc[:nt, slot, :],
                                start=(t == 0 and c == 0),
                                stop=False,
                                skip_group_check=True,
                            )
                        nc.tensor.matmul(
                            pet[:K, DC * K:DC * K + 1],
                            esc[:nt, slot, :],
                            ones_sb[:nt, :],
                            start=False,
                            stop=(t == NT - 1),
                            skip_group_check=True,
                        )

                def finalize(b, pet):
                    etf = finp.tile([128, 4, K], FP, tag="etf", name="etf")
                    etf_flat = bass.AP(
                        tensor=etf[:, :, :].tensor,
                        offset=etf[:, :, :].offset,
                        ap=[list(etf[:, :, :].ap[0]), [1, DC * K]],
                    )
                    nc.vector.tensor_copy(
                        out=etf_flat, in_=pet[:, :DC * K]
                    )
                    pef = pef_p.tile([K, 4, 128], FP, tag="pef", name="pef")
                    for c in range(DC):
                        nc.tensor.matmul(
                            pef[:, c, :],
                            etf[:, c, :],
                            identf[:, :],
                            start=True,
                            stop=True,
                            is_transpose=True,
                        )
                    e_sb = finp.tile([K, D], FP, tag="e_sb", name="e_sb")
                    pef_flat = bass.AP(
                        tensor=pef[:, :, :].tensor,
                        offset=pef[:, :, :].offset,
                        ap=[list(pef[:, :, :].ap[0]), [1, D]],
                    )
                    nc.vector.scalar_tensor_tensor(
                        out=e_sb,
                        in0=cneg_sb,
                        scalar=pet[:K, DC * K:DC * K + 1],
                        in1=pef_flat,
                        op0=ALU.mult,
                        op1=ALU.add,
                    )
                    nc.scalar.dma_start(out=e_d[b, :, :], in_=e_sb)

                # software pipeline: back-stage trails front-stage by SKEW
                # slabs; softmax runs per half-quad, emitted before copies so
                # it is not queued behind them on ACT/DVE
                pets = {}
                SKEW = 3
                for i in range(len(steps) + SKEW):
                    if i < len(steps):
                        b, s = steps[i]
                        if s == 0:
                            pets[b] = pet_p.tile(
                                [128, DC * K + 1], FP, tag="pet", name="pet"
                            )
                        stage_front(b, s)
                        if stages in ("full", "front", "smonly") and (
                            s % 4 in (1, 3)
                        ):
                            half_softmax(b, s // 4, (s % 4) // 2)
                        if stages in ("full", "front", "cponly"):
                            emit_copies(b, s)
                    j = i - SKEW
                    if stages != "full":
                        continue
                    if j >= 0:
                        b, s = steps[j]
                        stage_back(b, s, pets[b])
                        if s % 4 == 3 or s == NSLAB - 1:
                            escs.pop((b, s // 4))
                        if s == NSLAB - 1:
                            finalize(b, pets[b])

            for _rep in range(reps):
                emit_rep()

    return nc


# ---------------------------------------------------------------------------
# host side


def _host_inputs(X, codewords, scale):
    X = np.asarray(X, dtype=np.float32)
    codewords = np.asarray(codewords, dtype=np.float32)
    scale = np.asarray(scale, dtype=np.float32)

    Xr = X.reshape(B, D, N)
    x2 = np.einsum("bdn,bdn->bn", Xr, Xr)  # (B, N) fp32
    smax = scale.max()
    c2 = (codewords.astype(np.float64) ** 2).sum(axis=1).astype(np.float32)

    xin = np.ascontiguousarray(
        Xr.reshape(B, DC, 128, N)
    ).astype(ml_dtypes.bfloat16)
    aug = np.empty((B, 2, N), dtype=ml_dtypes.bfloat16)
    aug[:, 0, :] = x2.astype(ml_dtypes.bfloat16)
    aug[:, 1, :] = np.ones((B, N), dtype=ml_dtypes.bfloat16)
    rsl = np.ascontiguousarray(
        (-2.0 * scale[:, None] * codewords).T
    ).astype(ml_dtypes.bfloat16)  # (D, K)
    aug2 = np.stack([scale - smax, scale * c2]).astype(ml_dtypes.bfloat16)
    cneg = np.ascontiguousarray(-codewords)

    in_maps = []
    for c in range(N_CORES):
        in_maps.append(
            {
                "x": xin[c * BPC:(c + 1) * BPC],
                "aug": aug[c * BPC:(c + 1) * BPC],
                "rsl": rsl,
                "aug2": aug2,
                "cneg": cneg,
            }
        )
    return in_maps


class Runner:
    """jit-once / call-many executor for the SPMD kernel on 8 cores."""

    def __init__(self, reps: int = 1):
        import jax
        import numpy as np
        from jax.sharding import Mesh, NamedSharding, PartitionSpec
        from jax.experimental.shard_map import shard_map

        from concourse import bass2jax

        self.jax = jax
        nc = build(reps)
        bass2jax.install_neuronx_cc_hook()

        partition_name = (
            nc.partition_id_tensor.name if nc.partition_id_tensor else None
        )
        in_names, out_names, out_avals, zero_outs = [], [], [], []
        for alloc in nc.m.functions[0].allocations:
            if not isinstance(alloc, mybir.MemoryLocationSet):
                continue
            name = alloc.memorylocations[0].name
            if alloc.kind == "ExternalInput":
                if name != partition_name:
                    in_names.append(name)
            elif alloc.kind == "ExternalOutput":
                shape = tuple(alloc.tensor_shape)
                dt = mybir.dt.np(alloc.dtype)
                out_names.append(name)
                out_avals.append(
                    jax.core.ShapedArray(shape, dt)
                )
                zero_outs.append(np.zeros(shape, dt))
        self.in_names = list(in_names)
        self.out_names = out_names
        self.n_params = len(in_names)
        all_in_names = in_names + out_names
        if partition_name is not None:
            all_in_names.append(partition_name)

        def _body(*args):
            operands = list(args)
            if partition_name is not None:
                operands.append(bass2jax.partition_id_tensor())
            outs = bass2jax._bass_exec_p.bind(
                *operands,
                out_avals=tuple(out_avals),
                in_names=tuple(all_in_names),
                out_names=tuple(out_names),
                lowering_input_output_aliases=(),
                sim_require_finite=True,
                sim_require_nnan=True,
                nc=nc,
            )
            return tuple(outs)

        devices = jax.devices()[:N_CORES]
        self.mesh = Mesh(np.asarray(devices), ("core",))
        nin = self.n_params + len(out_names)
        self.fn = jax.jit(
            shard_map(
                _body,
                mesh=self.mesh,
                in_specs=(PartitionSpec("core"),) * nin,
                out_specs=(PartitionSpec("core"),) * len(out_names),
                check_rep=False,
            ),
            keep_unused=True,
        )
        self.sharding = NamedSharding(self.mesh, PartitionSpec("core"))
        self.zero_outs = zero_outs
        self._dev_args = None

    def put(self, in_maps):
        import jax

        concat = [
            np.concatenate([np.asarray(m[name]) for m in in_maps], axis=0)
            for name in self.in_names
        ]
        concat += [
            np.zeros((N_CORES * z.shape[0], *z.shape[1:]), z.dtype)
            for z in self.zero_outs
        ]
        self._dev_args = [jax.device_put(a, self.sharding) for a in concat]

    def run(self):
        outs = self.fn(*self._dev_args)
        self.jax.block_until_ready(outs)
        return outs

    def run_numpy(self):
        outs = self.run()
        res = []
        for c in range(N_CORES):
            res.append(
                {
                    name: np.asarray(outs[i]).reshape(
                        N_CORES, *self.zero_outs[i].shape
                    )[c]
                    for i, name in enumerate(self.out_names)
                }
            )
        return res


_RUNNER = None


def kernel(**inputs) -> np.ndarray:
    global _RUNNER
    X = np.asarray(inputs["X"], dtype=np.float32)
    codewords = np.asarray(inputs["codewords"], dtype=np.float32)
    scale = np.asarray(inputs["scale"], dtype=np.float32)
    if _RUNNER is None:
        _RUNNER = Runner(reps=1)
    _RUNNER.put(_host_inputs(X, codewords, scale))
    res = _RUNNER.run_numpy()
    E = np.concatenate([res[c]["e"] for c in range(N_CORES)], axis=0)
    return E.astype(np.float32)
